# revision 16
# baseline (speedup 1.0000x reference)
"""Trainium2 Bass kernel for the FNO-SMM problem (nn_FNO_SMM_34488587387600).

Data-parallel over 8 NeuronCores: 2 batches per core. Key structure (v2):
  - V built once in V^T [n, m] layout (PE selection matmuls + single fused
    mod-wrap on DVE + ACT sin in pi-units); the V [m, n] layout for the
    inverse NUDFT is produced by PE transposes of V^T (no second trig pass).
  - Mode-mix weights stored dense (64x64 per mode, even/odd halves) and
    DMA'd one half-layer slab at a time, double-buffered ahead of use.
  - Layer l+1's forward NUDFT is emitted inside layer l's inverse loop so
    the PE streams continuously; the fc1/fc2 head is fused into layer 3.
"""
import sys
import os

sys.path.insert(0, '/opt/trn_rl_repo')

import numpy as np
from contextlib import ExitStack

import concourse.bass as bass
import concourse.tile as tile
from concourse import bacc, mybir
from concourse.bass_utils import run_bass_kernel_spmd

MODES = 12
C = 32
N = 4096
B = 16
NCORES = 8
BL = B // NCORES          # 2 batches per core
NW = 299                  # working-set rows: 288 + 11 unpaired (kx=-12, ky<0)
NWP = 304                 # padded
PI = float(np.pi)
PI_SAFE = float(np.pi * (1.0 - 1e-6))

F32 = mybir.dt.float32
F32R = mybir.dt.float32r
F16 = mybir.dt.float16
AF = mybir.ActivationFunctionType
ALU = mybir.AluOpType

TRACE = False             # test harness can set kernel.TRACE = True

_CACHE = {}


# --------------------------------------------------------------------------
# host-side index helpers (python ints only; used at build/marshal time)
# --------------------------------------------------------------------------
def _w_rows():
    """W-set V-row indices: m in [0,288) then the 11 unpaired rows."""
    return list(range(288)) + [24 * j + 12 for j in range(12, 23)]


def _cap(t_ap, row0, nrows, pairs, free_off):
    """Custom AP on a tile's underlying tensor: rows [row0, row0+nrows),
    free pattern `pairs` ([[step, count], ...]) at element offset free_off."""
    base = t_ap.ap
    pstep = base[0][0]
    return bass.AP(tensor=t_ap.tensor, offset=row0 * pstep + free_off + t_ap.offset,
                   ap=[[pstep, nrows]] + [list(p) for p in pairs])


# --------------------------------------------------------------------------
# device program
# --------------------------------------------------------------------------
def _build_program():
    nc = bacc.Bacc("TRN2", target_bir_lowering=False, debug=False,
                   num_devices=NCORES)

    din = {}
    def dram_in(name, shape, dt):
        din[name] = nc.dram_tensor(name, list(shape), dt, kind="ExternalInput").ap()
        return din[name]

    ck_d = dram_in('ck', [BL, 50, N], F32R)
    selT_d = dram_in('selT', [50, 608], F32R)
    fc0w_d = dram_in('fc0w', [2, C], F32R)
    fc0b_d = dram_in('fc0b', [C, 1], F32)
    # dense mode-mix weights: [layer, even/odd, 64, 36*256]
    mmw_d = dram_in('mmw', [4, 2, 64, 9216], F16)
    cwt_d = dram_in('cwt', [4, C, C], F16)
    cb_d = dram_in('cb', [4, C, 1], F32)
    fc1w_d = dram_in('fc1w', [C, 128], F16)
    fc1b_d = dram_in('fc1b', [128, 1], F32)
    fc2w_d = dram_in('fc2w', [128, 1], F16)
    i64_d = dram_in('i64', [C, C], F16)     # identity (hT transposes)
    i128_d = dram_in('i128', [128, 128], F16)  # identity (vinv transposes)
    is32_d = dram_in('is32', [C, C], F32)   # identity
    js32_d = dram_in('js32', [C, C], F32)   # flipped identity

    y_d = nc.dram_tensor('y', [BL, N], F16, kind="ExternalOutput").ap()

    with tile.TileContext(nc) as tc, ExitStack() as ctx:
        # ------------- persistent pool -------------
        pp = ctx.enter_context(tc.tile_pool(name="persist", bufs=1))
        vt = [[pp.tile([128, 608], F16, tag=f"vt{b}_{c}", name=f"vt{b}_{c}")
               for c in range(32)] for b in range(BL)]
        vrows = [128, 128, 48, 128, 128, 48]
        vinv = [[pp.tile([vrows[k], N], F16, tag=f"vi{b}_{k}", name=f"vi{b}_{k}")
                 for k in range(6)] for b in range(BL)]

        def vinv_ap(b, k, cols):
            return vinv[b][k][:, cols]
        h = [pp.tile([C, N], F16, tag=f"h{b}", name=f"h{b}") for b in range(BL)]

        cwt_t = [pp.tile([C, C], F16, tag=f"cwt{l}", name=f"cwt{l}") for l in range(4)]
        cb_t = [pp.tile([C, 1], F32, tag=f"cb{l}", name=f"cb{l}") for l in range(4)]
        fc1w_t = pp.tile([C, 128], F16, tag="fc1w", name="fc1w_t")
        fc1b_t = pp.tile([128, 1], F32, tag="fc1b", name="fc1b_t")
        fc2w_t = pp.tile([128, 1], F16, tag="fc2w", name="fc2w_t")
        i64_t = pp.tile([C, C], F16, tag="i64", name="i64_t")
        is32_t = pp.tile([C, C], F32, tag="is32", name="is32_t")
        js32_t = pp.tile([C, C], F32, tag="js32", name="js32_t")
        # mode-mix eighth-layer slab ring (2 bufs): [128, 18*64]
        mslab = [pp.tile([128, 1152], F16, tag=f"ms{i}", name=f"ms{i}")
                 for i in range(2)]

        for l in range(4):
            nc.sync.dma_start(cwt_t[l][:], cwt_d[l])
            nc.sync.dma_start(cb_t[l][:], cb_d[l])
        nc.sync.dma_start(fc1w_t[:], fc1w_d[:])
        nc.sync.dma_start(fc1b_t[:], fc1b_d[:])
        nc.sync.dma_start(fc2w_t[:], fc2w_d[:])
        nc.sync.dma_start(i64_t[:], i64_d[:])
        nc.sync.dma_start(is32_t[:], is32_d[:])
        nc.sync.dma_start(js32_t[:], js32_d[:])

        def mm_dma(g):
            """Fetch mode-mix eighth-layer slab g (= 8*l + q) into ring buf."""
            l, qq = g // 8, g % 8
            sl = mslab[g % 2]
            cols = slice(1152 * qq, 1152 * (qq + 1))
            nc.sync.dma_start(sl[0:64, :], mmw_d[l, 0, :, cols])
            nc.sync.dma_start(sl[64:128, :], mmw_d[l, 1, :, cols])

        mm_dma(0)
        mm_dma(1)

        # ---------------- shared PSUM pool (phases A+B) ----------------
        abps = ctx.enter_context(tc.tile_pool(name="abps", bufs=1, space="PSUM"))
        px = abps.tile([128, NWP], F32, tag="px", name="px")

        # ---------------- forward NUDFT emitter ----------------
        wk_pools = {}

        def fwd_chunk(l, c8, hpool, hps):
            """Forward-NUDFT contribution of n-chunk c8 (both batches) into px."""
            for s in range(4):
                kt = 4 * c8 + s
                pt = hps.tile([128, 2 * C], F16, tag="pt", bufs=1,
                              name=f"pt{l}_{kt}")
                for b in range(BL):
                    nc.tensor.matmul(pt[:, 32 * b:32 * (b + 1)],
                                     h[b][:, 128 * kt:128 * (kt + 1)],
                                     i64_t[:], start=True, stop=True,
                                     is_transpose=True)
                hTt = hpool.tile([128, 2 * C], F16, tag="hT", bufs=3,
                                 name=f"hT{l}_{kt}")
                # fp16 PE-transpose ignores the stationary operand, so the
                # 1/64 NUDFT pre-scale is applied here instead
                nc.vector.tensor_scalar(hTt[:], pt[:], 1.0 / 64.0, None,
                                        op0=ALU.mult)
                for g in range(4):
                    b, ri = g // 2, g % 2      # ri: 0 = real, 1 = imag
                    rhs = vt[b][kt][:, 304:608] if ri == 0 else vt[b][kt][:, 0:304]
                    nc.tensor.matmul(px[32 * g:32 * (g + 1), :],
                                     hTt[:, 32 * b:32 * (b + 1)], rhs,
                                     start=(kt == 0), stop=(kt == 31),
                                     tile_position=(0, 32 * g))

        # ------------- phase A: V build + fc0 + layer-0 forward -------------
        with tc.tile_pool(name="vbuild", bufs=1) as vb, \
             tc.tile_pool(name="vbps", bufs=1, space="PSUM") as vbps:
            fc0w_t = vb.tile([2, C], F32R, tag="fc0w", name="fc0w_t")
            fc0b_t = vb.tile([C, 1], F32, tag="fc0b", name="fc0b_t")
            i128_t = vb.tile([128, 128], F16, tag="i128", name="i128_t")
            selT_t = vb.tile([50, 608], F32R, tag="selT", name="selT_t")
            nc.sync.dma_start(fc0w_t[:], fc0w_d[:])
            nc.sync.dma_start(fc0b_t[:], fc0b_d[:])
            nc.sync.dma_start(i128_t[:], i128_d[:])
            nc.sync.dma_start(selT_t[:], selT_d[:])
            for c8 in range(8):
                cols = slice(512 * c8, 512 * (c8 + 1))
                for b in range(BL):
                    ckt = vb.tile([50, 512], F32R, tag="ck", bufs=2,
                                  name=f"ck{b}_{c8}")
                    nc.sync.dma_start(ckt[:], ck_d[b, :, cols])

                    # fc0 for this chunk
                    ph0 = vbps.tile([C, 512], F32, tag="ph0", bufs=1,
                                    name=f"ph0_{b}_{c8}")
                    nc.tensor.matmul(ph0[:], fc0w_t[:], ckt[0:2, :],
                                     start=True, stop=True)
                    nc.scalar.activation(h[b][:, cols], ph0[:], AF.Identity,
                                         bias=fc0b_t[:, :])

                    # V^T slabs for the 4 n-subchunks of 128.  The selection
                    # matmul emits u = (theta + shift)/pi + 1 directly (const
                    # row of ck); one fused mod-wrap maps u to [-1, 1); Sin
                    # with scale pi recovers sin(theta + shift).
                    for s in range(4):
                        pva = vbps.tile([128, 512], F32, tag="pva", bufs=2,
                                        name=f"pva{b}_{c8}_{s}")
                        pvb = vbps.tile([128, 96], F32, tag="pvb", bufs=1,
                                        name=f"pvb{b}_{c8}_{s}")
                        lhs = ckt[:, 128 * s:128 * (s + 1)]
                        nc.tensor.matmul(pva[:], lhs, selT_t[:, 0:512],
                                         start=True, stop=True)
                        nc.tensor.matmul(pvb[:], lhs, selT_t[:, 512:608],
                                         start=True, stop=True)
                        nc.vector.add_range_wrap(pva[:], pva[:], shift=0.0,
                                                 bound=PI, period=2 * PI)
                        nc.vector.add_range_wrap(pvb[:], pvb[:], shift=0.0,
                                                 bound=PI, period=2 * PI)
                        nc.scalar.activation(vt[b][4 * c8 + s][:, 0:512],
                                             pva[:], AF.Sin)
                        nc.scalar.activation(vt[b][4 * c8 + s][:, 512:608],
                                             pvb[:], AF.Sin)

                # layer-0 forward NUDFT for this n-chunk
                fwd_chunk(0, c8, vb, vbps)

                # V [m, n] tiles by transposing the finished V^T chunk
                for b in range(BL):
                    for k in range(6):
                        w = vrows[k]
                        c0 = (0 if k >= 3 else 304) + 128 * (k % 3)
                        ptr = vbps.tile([128, 512], F16, tag="ptr", bufs=2,
                                        name=f"ptr{b}_{c8}_{k}")
                        for s in range(4):
                            nc.tensor.matmul(
                                ptr[0:w, 128 * s:128 * (s + 1)],
                                vt[b][4 * c8 + s][:, c0:c0 + w],
                                i128_t[:], start=True, stop=True,
                                is_transpose=True)
                        if k < 3:
                            nc.vector.tensor_copy(vinv_ap(b, k, cols),
                                                  ptr[0:w, :])
                        else:
                            nc.scalar.activation(vinv_ap(b, k, cols),
                                                 ptr[0:w, :], AF.Copy)

        # ------------- phase B: layers -------------
        with tc.tile_pool(name="work", bufs=1) as wk, \
             tc.tile_pool(name="wkps", bufs=1, space="PSUM") as wkps:

            for l in range(4):
                # ---- R slab (mode-mix inputs), both batches interleaved ----
                R = wk.tile([128, 288], F16, tag="R", bufs=1, name=f"R{l}")
                R3 = R.rearrange("p (a s) -> p a s", s=12)
                for b in range(BL):
                    row_xr = 64 * b          # px rows: g = 2b + ri
                    row_xi = 64 * b + 32
                    for par in range(2):
                        out_r0 = 0 if par == 0 else 64
                        out_i0 = 32 if par == 0 else 96
                        # top + a=12 (direct): m = 23a + 2q + par, a in [0,12]
                        nc.vector.tensor_copy(
                            _cap(R3, out_r0, 32, [[12, 13], [2, 6]], b),
                            _cap(px, row_xr, 32, [[23, 13], [2, 6]], par))
                        nc.vector.tensor_copy(
                            _cap(R3, out_i0, 32, [[12, 13], [2, 6]], b),
                            _cap(px, row_xi, 32, [[23, 13], [2, 6]], par))
                        # bot bulk (conj): a in [13,24), in col 576-23a-2q-par
                        nc.vector.tensor_copy(
                            _cap(R3, out_r0, 32, [[12, 11], [2, 6]], 156 + b),
                            _cap(px, row_xr, 32, [[-23, 11], [-2, 6]], 277 - par))
                        nc.vector.tensor_scalar(
                            _cap(R3, out_i0, 32, [[12, 11], [2, 6]], 156 + b),
                            _cap(px, row_xi, 32, [[-23, 11], [-2, 6]], 277 - par),
                            -1.0, None, op0=ALU.mult)
                        # fixups: s = a-12 (P-columns, direct, xi positive)
                        cnt = 5 if par == 0 else 6
                        s0 = 2 if par == 0 else 1
                        o_off = 13 * s0 + 144 - par + b
                        i_off = 288 + s0 - 1
                        nc.vector.tensor_copy(
                            _cap(R3, out_r0, 32, [[26, cnt]], o_off),
                            _cap(px, row_xr, 32, [[2, cnt]], i_off))
                        nc.vector.tensor_copy(
                            _cap(R3, out_i0, 32, [[26, cnt]], o_off),
                            _cap(px, row_xi, 32, [[2, cnt]], i_off))

                # ---- mode mix: 288 64x64 matmuls (even/odd quadrants) ----
                pm = wkps.tile([128, 288], F32, tag="pm", bufs=1, name=f"pm{l}")
                for qq in range(8):
                    sl = mslab[(8 * l + qq) % 2]
                    for rl in range(18):
                        r = 18 * qq + rl
                        wcols = slice(64 * rl, 64 * rl + 64)
                        nc.tensor.matmul(pm[0:64, 2 * r:2 * r + 2],
                                         sl[0:64, wcols],
                                         R[0:64, 2 * r:2 * r + 2],
                                         start=True, stop=True,
                                         tile_position=(0, 0))
                        nc.tensor.matmul(pm[64:128, 2 * r:2 * r + 2],
                                         sl[64:128, wcols],
                                         R[64:128, 2 * r:2 * r + 2],
                                         start=True, stop=True,
                                         tile_position=(64, 64))
                    # prefetch the slab two eighths ahead
                    g_next = 8 * l + qq + 2
                    if g_next < 32:
                        mm_dma(g_next)

                # ---- flat extraction + coefficient slabs ----
                frs = [wk.tile([C, NWP], F32, tag=f"frs{b}", name=f"frs{l}_{b}")
                       for b in range(BL)]
                fis = [wk.tile([C, NWP], F32, tag=f"fis{b}", name=f"fis{l}_{b}")
                       for b in range(BL)]
                frx = [wk.tile([C, NWP], F32, tag=f"frx{b}", name=f"frx{l}_{b}")
                       for b in range(BL)]
                fix = [wk.tile([C, NWP], F32, tag=f"fix{b}", name=f"fix{l}_{b}")
                       for b in range(BL)]
                for b in range(BL):
                    nc.vector.memset(frs[b][:, 288:NWP], 0.0)
                    nc.vector.memset(fis[b][:, 288:NWP], 0.0)
                    nc.vector.memset(frx[b][:], 0.0)
                    nc.vector.memset(fix[b][:], 0.0)
                    # even u from pm rows 0:32 (or) / 32:64 (oi), odd u from 64:96 / 96:128
                    nc.vector.tensor_copy(_cap(frs[b], 0, 32, [[2, 144]], 0),
                                          _cap(pm, 0, 32, [[2, 144]], b))
                    nc.vector.tensor_copy(_cap(frs[b], 0, 32, [[2, 144]], 1),
                                          _cap(pm, 64, 32, [[2, 144]], b))
                    nc.vector.tensor_copy(_cap(fis[b], 0, 32, [[2, 144]], 0),
                                          _cap(pm, 32, 32, [[2, 144]], b))
                    nc.vector.tensor_copy(_cap(fis[b], 0, 32, [[2, 144]], 1),
                                          _cap(pm, 96, 32, [[2, 144]], b))
                    # frx/fix: partner-coefficient slabs (read via rearranged views)
                    for (dst, src) in ((frx[b], frs[b]), (fix[b], fis[b])):
                        d3 = dst[:, 0:288].rearrange("p (j i) -> p j i", i=24)
                        s3 = src[:, 0:288].rearrange("p (j i) -> p j i", i=24)
                        nc.vector.tensor_copy(d3[:, 1:12, 1:12], s3[:, 1:12, 0:11])
                        nc.vector.tensor_copy(d3[:, 1:12, 13:24], s3[:, 1:12, 12:23])
                        nc.vector.tensor_copy(d3[:, 1:12, 0:1], s3[:, 1:12, 23:24])
                        # P columns: partner col 24*(23-j2)+11, j2 = 12..22
                        nc.vector.tensor_copy(dst[:, 288:299],
                                              s3[:, 11:0:-1, 11:12].rearrange(
                                                  "p j i -> p (j i)"))
                    nc.vector.tensor_scalar(fix[b][:, 288:299], fix[b][:, 288:299],
                                            -1.0, None, op0=ALU.mult)

                # ---- A^T / B^T via accumulate transposes ----
                cw3 = [128, 128, 48]
                AT = [[wk.tile([cw3[ch], C], F16, tag=f"AT{b}_{ch}",
                               name=f"AT{l}_{b}_{ch}") for ch in range(3)]
                      for b in range(BL)]
                BT = [[wk.tile([cw3[ch], C], F16, tag=f"BT{b}_{ch}",
                               name=f"BT{l}_{b}_{ch}") for ch in range(3)]
                      for b in range(BL)]
                for b in range(BL):
                    for ch in range(3):
                        cw_ = cw3[ch]
                        csl = slice(128 * ch, 128 * ch + cw_)
                        for di, (dstt, s_dir, s_flp) in enumerate(
                                ((AT[b][ch], frs[b], frx[b]),
                                 (BT[b][ch], fis[b], fix[b]))):
                            pc = wkps.tile([128, C], F32, tag="pc", bufs=1,
                                           name=f"pc{l}_{b}_{ch}_{di}")
                            nc.tensor.matmul(pc[0:cw_, :], s_dir[:, csl], is32_t[:],
                                             start=True, stop=False,
                                             is_transpose=True)
                            nc.tensor.matmul(pc[0:cw_, :], s_flp[:, csl], js32_t[:],
                                             start=False, stop=True,
                                             is_transpose=True)
                            # transpose-mode rhs magnitudes are not applied;
                            # the 1/32 coefficient scale happens here
                            nc.vector.tensor_scalar(dstt[:], pc[0:cw_, :],
                                                    1.0 / 32.0, None, op0=ALU.mult)

                # ---- inverse NUDFT + conv + activation (+ next-layer fwd) ----
                last = (l == 3)
                for c8 in range(8):
                    cols = slice(512 * c8, 512 * (c8 + 1))
                    pi_ = wkps.tile([64, 512], F32, tag="pinv", bufs=2,
                                    name=f"pinv{l}_{c8}")
                    for b in range(BL):
                        sl = pi_[32 * b:32 * (b + 1), :]
                        tp = (0, 32 * b)
                        nc.tensor.matmul(sl, AT[b][0][:], vinv_ap(b, 0, cols),
                                         start=True, stop=False, tile_position=tp)
                        nc.tensor.matmul(sl, AT[b][1][:], vinv_ap(b, 1, cols),
                                         start=False, stop=False, tile_position=tp)
                        nc.tensor.matmul(sl, AT[b][2][:], vinv_ap(b, 2, cols),
                                         start=False, stop=False, tile_position=tp)
                        nc.tensor.matmul(sl, BT[b][0][:], vinv_ap(b, 3, cols),
                                         start=False, stop=False, tile_position=tp)
                        nc.tensor.matmul(sl, BT[b][1][:], vinv_ap(b, 4, cols),
                                         start=False, stop=False, tile_position=tp)
                        nc.tensor.matmul(sl, BT[b][2][:], vinv_ap(b, 5, cols),
                                         start=False, stop=False, tile_position=tp)
                        nc.tensor.matmul(sl, cwt_t[l][:], h[b][:, cols],
                                         start=False, stop=True, tile_position=tp)
                    for b in range(BL):
                        nc.scalar.activation(
                            h[b][:, cols], pi_[32 * b:32 * (b + 1), :],
                            AF.Identity if last else AF.Gelu,
                            bias=cb_t[l][:, :])
                    if not last:
                        fwd_chunk(l + 1, c8, wk, wkps)
                    else:
                        # ---- head: fc1 + gelu + fc2, fused per chunk ----
                        for b in range(BL):
                            pg = wkps.tile([128, 512], F32, tag="pg", bufs=1,
                                           name=f"pg{b}_{c8}")
                            nc.tensor.matmul(pg[:], fc1w_t[:], h[b][:, cols],
                                             start=True, stop=True)
                            g = wk.tile([128, 512], F16, tag="g", bufs=2,
                                        name=f"g{b}_{c8}")
                            nc.scalar.activation(g[:], pg[:], AF.Gelu,
                                                 bias=fc1b_t[:, :])
                            py = wkps.tile([1, 512], F32, tag="py", bufs=1,
                                           name=f"py{b}_{c8}")
                            nc.tensor.matmul(py[:], fc2w_t[:], g[:],
                                             start=True, stop=True)
                            ys = wk.tile([1, 512], F16, tag="ys", bufs=1,
                                         name=f"ys{b}_{c8}")
                            nc.scalar.activation(ys[:], py[:], AF.Copy)
                            nc.sync.dma_start(y_d[b:b + 1, cols], ys[:])

    nc.compile()
    return nc


# --------------------------------------------------------------------------
# host marshaling
# --------------------------------------------------------------------------
def _marshal(pos, fc0_w, fc0_b, sw1r, sw1i, sw2r, sw2i, cw, cb,
             fc1_w, fc1_b, fc2_w, fc2_b):
    xp = (pos[:, :, 0] - pos[:, :, 0].min()).astype(np.float64)
    yp = (pos[:, :, 1] - pos[:, :, 1].min()).astype(np.float64)
    sx = np.float64(np.float32(6.28) / np.float32(xp.max()))
    sy = np.float64(np.float32(6.28) / np.float32(yp.max()))
    kx = np.concatenate([np.arange(MODES), np.arange(-MODES, 0)]).astype(np.float64)
    ky = np.concatenate([np.arange(MODES), np.arange(-(MODES - 1), 0)]).astype(np.float64)

    def wrap(v):
        return v - 2 * np.pi * np.round(v / (2 * np.pi))

    ck = np.zeros((B, 50, N), np.float32)
    ck[:, 0, :] = xp.astype(np.float32)
    ck[:, 1, :] = yp.astype(np.float32)
    for i in range(24):
        ck[:, 2 + i, :] = wrap(kx[i] * sx * xp).astype(np.float32)
    for j in range(23):
        ck[:, 26 + j, :] = wrap(ky[j] * sy * yp).astype(np.float32)
    ck[:, 49, :] = 1.0

    worder = _w_rows()
    # selT [50, 608]: cols 0:304 = -phase (Vi, sin(-theta)); cols 304:608 =
    # +phase (Vr, cos); the pi/2 cos shift is folded into the constant row so
    # one range-wrap covers every column.
    selT = np.zeros((50, 608), np.float32)
    for w, m in enumerate(worder):
        i, j = m % 24, m // 24
        selT[2 + i, w] = -1.0
        selT[26 + j, w] = -1.0
        selT[2 + i, 304 + w] = 1.0
        selT[26 + j, 304 + w] = 1.0
    selT[49, 0:304] = 0.0
    selT[49, 304:608] = np.pi / 2

    # mode-mix weights, dense augmented 64x64 blocks, even/odd halves
    mmw = np.zeros((4, 2, 64, 9216), np.float16)
    for l in range(4):
        w1 = sw1r[l].astype(np.float64) + 1j * sw1i[l].astype(np.float64)
        w2 = sw2r[l].astype(np.float64) + 1j * sw2i[l].astype(np.float64)
        for u in range(288):
            a, s = u // 12, u % 12
            wm = w1[:, :, a, s] if a < 12 else w2[:, :, a - 12, s]
            wr = wm.real.astype(np.float16)
            wi = wm.imag.astype(np.float16)
            r, par = u // 2, u % 2
            blk = np.zeros((64, 64), np.float16)
            blk[0:32, 0:32] = wr
            blk[32:64, 0:32] = -wi
            blk[0:32, 32:64] = wi
            blk[32:64, 32:64] = wr
            mmw[l, par, :, 64 * r:64 * (r + 1)] = blk

    cwt = np.ascontiguousarray(cw.transpose(0, 2, 1)).astype(np.float16)  # [l, c_in, c_out]
    cbm = cb.reshape(4, C, 1).astype(np.float32)

    eye = np.eye(C, dtype=np.float32)
    args = dict(
        selT=selT,
        fc0w=fc0_w.astype(np.float32), fc0b=fc0_b.reshape(C, 1).astype(np.float32),
        mmw=mmw, cwt=cwt, cb=cbm,
        fc1w=fc1_w.astype(np.float16), fc1b=fc1_b.reshape(128, 1).astype(np.float32),
        fc2w=fc2_w.reshape(128, 1).astype(np.float16),
        i64=eye.astype(np.float16),
        i128=np.eye(128, dtype=np.float16),
        is32=eye.astype(np.float32),
        js32=eye[::-1].copy().astype(np.float32),
    )
    return ck, args


def kernel(**inputs):
    pos = np.asarray(inputs['pos'])
    ck, shared = _marshal(**{k: np.asarray(v) for k, v in inputs.items()})

    if 'nc' not in _CACHE:
        _CACHE['nc'] = _build_program()
    nc = _CACHE['nc']

    in_maps = []
    for core in range(NCORES):
        m = dict(shared)
        m['ck'] = ck[BL * core:BL * (core + 1)]
        in_maps.append(m)

    res = run_bass_kernel_spmd(nc, in_maps, list(range(NCORES)), trace=TRACE)
    _CACHE['last_results'] = res

    fc2_b = np.asarray(inputs['fc2_b']).astype(np.float32)
    out = np.zeros((B, N, 1), np.float32)
    for core in range(NCORES):
        out[BL * core:BL * (core + 1), :, 0] = res.results[core]['y'].astype(np.float32)
    out += fc2_b.reshape(1, 1, 1)
    return out


# revision 23
# speedup vs baseline: 1.0385x; 1.0385x over previous
"""Trainium2 Bass kernel for the FNO-SMM problem (nn_FNO_SMM_34488587387600).

Data-parallel over 8 NeuronCores: 2 batches per core. Key structure (v2):
  - V built once in V^T [n, m] layout (PE selection matmuls + single fused
    mod-wrap on DVE + ACT sin in pi-units); the V [m, n] layout for the
    inverse NUDFT is produced by PE transposes of V^T (no second trig pass).
  - Mode-mix weights stored dense (64x64 per mode, even/odd halves) and
    DMA'd one half-layer slab at a time, double-buffered ahead of use.
  - Layer l+1's forward NUDFT is emitted inside layer l's inverse loop so
    the PE streams continuously; the fc1/fc2 head is fused into layer 3.
"""
import sys
import os

sys.path.insert(0, '/opt/trn_rl_repo')

import numpy as np
from contextlib import ExitStack

import concourse.bass as bass
import concourse.tile as tile
from concourse import bacc, mybir
from concourse.bass_utils import run_bass_kernel_spmd

MODES = 12
C = 32
N = 4096
B = 16
NCORES = 8
BL = B // NCORES          # 2 batches per core
NW = 299                  # working-set rows: 288 + 11 unpaired (kx=-12, ky<0)
NWP = 304                 # padded
PI = float(np.pi)
PI_SAFE = float(np.pi * (1.0 - 1e-6))

F32 = mybir.dt.float32
F32R = mybir.dt.float32r
F16 = mybir.dt.float16
AF = mybir.ActivationFunctionType
ALU = mybir.AluOpType

TRACE = False             # test harness can set kernel.TRACE = True

_CACHE = {}


# --------------------------------------------------------------------------
# host-side index helpers (python ints only; used at build/marshal time)
# --------------------------------------------------------------------------
def _w_rows():
    """W-set V-row indices: m in [0,288) then the 11 unpaired rows."""
    return list(range(288)) + [24 * j + 12 for j in range(12, 23)]


def _cap(t_ap, row0, nrows, pairs, free_off):
    """Custom AP on a tile's underlying tensor: rows [row0, row0+nrows),
    free pattern `pairs` ([[step, count], ...]) at element offset free_off."""
    base = t_ap.ap
    pstep = base[0][0]
    return bass.AP(tensor=t_ap.tensor, offset=row0 * pstep + free_off + t_ap.offset,
                   ap=[[pstep, nrows]] + [list(p) for p in pairs])


# --------------------------------------------------------------------------
# device program
# --------------------------------------------------------------------------
def _build_program():
    nc = bacc.Bacc("TRN2", target_bir_lowering=False, debug=False,
                   num_devices=NCORES)

    din = {}
    def dram_in(name, shape, dt):
        din[name] = nc.dram_tensor(name, list(shape), dt, kind="ExternalInput").ap()
        return din[name]

    ck_d = dram_in('ck', [BL, 50, N], F32R)
    selT_d = dram_in('selT', [50, 608], F32R)
    fc0w_d = dram_in('fc0w', [2, C], F32R)
    fc0b_d = dram_in('fc0b', [C, 1], F32)
    # dense mode-mix weights: [layer, 128, 144*64] (rows 0:64 even modes,
    # rows 64:128 odd modes)
    mmw_d = dram_in('mmw', [4, 128, 9216], F16)
    cwt_d = dram_in('cwt', [4, C, C], F16)
    cb_d = dram_in('cb', [4, C, 1], F32)
    fc1w_d = dram_in('fc1w', [C, 128], F16)
    fc1b_d = dram_in('fc1b', [128, 1], F32)
    fc2w_d = dram_in('fc2w', [128, 1], F16)
    i64_d = dram_in('i64', [C, C], F16)     # identity (hT transposes)
    i128_d = dram_in('i128', [128, 128], F16)  # identity (vinv transposes)
    is32_d = dram_in('is32', [C, C], F32)   # identity
    js32_d = dram_in('js32', [C, C], F32)   # flipped identity

    y_d = nc.dram_tensor('y', [BL, N], F16, kind="ExternalOutput").ap()

    with tile.TileContext(nc) as tc, ExitStack() as ctx:
        # ------------- persistent pool -------------
        pp = ctx.enter_context(tc.tile_pool(name="persist", bufs=1))
        vt = [[pp.tile([128, 608], F16, tag=f"vt{b}_{c}", name=f"vt{b}_{c}")
               for c in range(32)] for b in range(BL)]
        vrows = [128, 128, 48, 128, 128, 48]
        vinv = [[pp.tile([vrows[k], N], F16, tag=f"vi{b}_{k}", name=f"vi{b}_{k}")
                 for k in range(6)] for b in range(BL)]

        def vinv_ap(b, k, cols):
            return vinv[b][k][:, cols]
        h = [pp.tile([C, N], F16, tag=f"h{b}", name=f"h{b}") for b in range(BL)]

        cwt_t = [pp.tile([C, C], F16, tag=f"cwt{l}", name=f"cwt{l}") for l in range(4)]
        cb_t = [pp.tile([C, 1], F32, tag=f"cb{l}", name=f"cb{l}") for l in range(4)]
        fc1w_t = pp.tile([C, 128], F16, tag="fc1w", name="fc1w_t")
        fc1b_t = pp.tile([128, 1], F32, tag="fc1b", name="fc1b_t")
        fc2w_t = pp.tile([128, 1], F16, tag="fc2w", name="fc2w_t")
        i64_t = pp.tile([C, C], F16, tag="i64", name="i64_t")
        is32_t = pp.tile([C, C], F32, tag="is32", name="is32_t")
        js32_t = pp.tile([C, C], F32, tag="js32", name="js32_t")
        # mode-mix eighth-layer slab ring (2 bufs): [128, 18*64]
        mslab = [pp.tile([128, 1152], F16, tag=f"ms{i}", name=f"ms{i}")
                 for i in range(2)]

        for l in range(4):
            nc.sync.dma_start(cwt_t[l][:], cwt_d[l])
            nc.sync.dma_start(cb_t[l][:], cb_d[l])
        nc.sync.dma_start(fc1w_t[:], fc1w_d[:])
        nc.sync.dma_start(fc1b_t[:], fc1b_d[:])
        nc.sync.dma_start(fc2w_t[:], fc2w_d[:])
        nc.sync.dma_start(i64_t[:], i64_d[:])
        nc.sync.dma_start(is32_t[:], is32_d[:])
        nc.sync.dma_start(js32_t[:], js32_d[:])

        def mm_dma(g):
            """Fetch mode-mix eighth-layer slab g (= 8*l + q) into ring buf."""
            l, qq = g // 8, g % 8
            sl = mslab[g % 2]
            cols = slice(1152 * qq, 1152 * (qq + 1))
            nc.sync.dma_start(sl[:, :], mmw_d[l, :, cols])

        mm_dma(0)
        mm_dma(1)

        # ---------------- shared PSUM pool (phases A+B) ----------------
        abps = ctx.enter_context(tc.tile_pool(name="abps", bufs=1, space="PSUM"))
        px = abps.tile([128, NWP], F32, tag="px", name="px")

        # ---------------- forward NUDFT emitter ----------------
        wk_pools = {}

        def fwd_chunk(l, c8, hpool, hps):
            """Forward-NUDFT contribution of n-chunk c8 (both batches) into px."""
            for s in range(4):
                kt = 4 * c8 + s
                pt = hps.tile([128, 2 * C], F16, tag="pt", bufs=1,
                              name=f"pt{l}_{kt}")
                for b in range(BL):
                    nc.tensor.matmul(pt[:, 32 * b:32 * (b + 1)],
                                     h[b][:, 128 * kt:128 * (kt + 1)],
                                     i64_t[:], start=True, stop=True,
                                     is_transpose=True)
                hTt = hpool.tile([128, 2 * C], F16, tag="hT", bufs=3,
                                 name=f"hT{l}_{kt}")
                # fp16 PE-transpose ignores the stationary operand, so the
                # 1/64 NUDFT pre-scale is applied here instead
                nc.vector.tensor_scalar(hTt[:], pt[:], 1.0 / 64.0, None,
                                        op0=ALU.mult)
                for g in range(4):
                    b, ri = g // 2, g % 2      # ri: 0 = real, 1 = imag
                    rhs = vt[b][kt][:, 304:608] if ri == 0 else vt[b][kt][:, 0:304]
                    nc.tensor.matmul(px[32 * g:32 * (g + 1), :],
                                     hTt[:, 32 * b:32 * (b + 1)], rhs,
                                     start=(kt == 0), stop=(kt == 31),
                                     tile_position=(0, 32 * g))

        # ------------- phase A: V build + fc0 + layer-0 forward -------------
        with tc.tile_pool(name="vbuild", bufs=1) as vb, \
             tc.tile_pool(name="vbps", bufs=1, space="PSUM") as vbps:
            fc0w_t = vb.tile([2, C], F32R, tag="fc0w", name="fc0w_t")
            fc0b_t = vb.tile([C, 1], F32, tag="fc0b", name="fc0b_t")
            i128_t = vb.tile([128, 128], F16, tag="i128", name="i128_t")
            selT_t = vb.tile([50, 608], F32R, tag="selT", name="selT_t")
            nc.sync.dma_start(fc0w_t[:], fc0w_d[:])
            nc.sync.dma_start(fc0b_t[:], fc0b_d[:])
            nc.sync.dma_start(i128_t[:], i128_d[:])
            nc.sync.dma_start(selT_t[:], selT_d[:])
            for c8 in range(8):
                cols = slice(512 * c8, 512 * (c8 + 1))
                for b in range(BL):
                    ckt = vb.tile([50, 512], F32R, tag="ck", bufs=2,
                                  name=f"ck{b}_{c8}")
                    nc.sync.dma_start(ckt[:], ck_d[b, :, cols])

                    # fc0 for this chunk
                    ph0 = vbps.tile([C, 512], F32, tag="ph0", bufs=1,
                                    name=f"ph0_{b}_{c8}")
                    nc.tensor.matmul(ph0[:], fc0w_t[:], ckt[0:2, :],
                                     start=True, stop=True)
                    nc.scalar.activation(h[b][:, cols], ph0[:], AF.Identity,
                                         bias=fc0b_t[:, :])

                    # V^T slabs for the 4 n-subchunks of 128.  The selection
                    # matmul emits u = (theta + shift)/pi + 1 directly (const
                    # row of ck); one fused mod-wrap maps u to [-1, 1); Sin
                    # with scale pi recovers sin(theta + shift).
                    for s in range(4):
                        pva = vbps.tile([128, 512], F32, tag="pva", bufs=2,
                                        name=f"pva{b}_{c8}_{s}")
                        pvb = vbps.tile([128, 96], F32, tag="pvb", bufs=1,
                                        name=f"pvb{b}_{c8}_{s}")
                        lhs = ckt[:, 128 * s:128 * (s + 1)]
                        nc.tensor.matmul(pva[:], lhs, selT_t[:, 0:512],
                                         start=True, stop=True)
                        nc.tensor.matmul(pvb[:], lhs, selT_t[:, 512:608],
                                         start=True, stop=True)
                        nc.vector.add_range_wrap(pva[:], pva[:], shift=0.0,
                                                 bound=PI, period=2 * PI)
                        nc.vector.add_range_wrap(pvb[:], pvb[:], shift=0.0,
                                                 bound=PI, period=2 * PI)
                        nc.scalar.activation(vt[b][4 * c8 + s][:, 0:512],
                                             pva[:], AF.Sin)
                        nc.scalar.activation(vt[b][4 * c8 + s][:, 512:608],
                                             pvb[:], AF.Sin)

                # layer-0 forward NUDFT for this n-chunk
                fwd_chunk(0, c8, vb, vbps)

                # V [m, n] tiles by transposing the finished V^T chunk
                for b in range(BL):
                    for k in range(6):
                        w = vrows[k]
                        c0 = (0 if k >= 3 else 304) + 128 * (k % 3)
                        ptr = vbps.tile([128, 512], F16, tag="ptr", bufs=2,
                                        name=f"ptr{b}_{c8}_{k}")
                        for s in range(4):
                            nc.tensor.matmul(
                                ptr[0:w, 128 * s:128 * (s + 1)],
                                vt[b][4 * c8 + s][:, c0:c0 + w],
                                i128_t[:], start=True, stop=True,
                                is_transpose=True)
                        if k < 4:
                            nc.vector.tensor_copy(vinv_ap(b, k, cols),
                                                  ptr[0:w, :])
                        else:
                            nc.scalar.activation(vinv_ap(b, k, cols),
                                                 ptr[0:w, :], AF.Copy)

        # ------------- phase B: layers -------------
        with tc.tile_pool(name="work", bufs=1) as wk, \
             tc.tile_pool(name="wkps", bufs=1, space="PSUM") as wkps:

            for l in range(4):
                # ---- R slab (mode-mix inputs), both batches interleaved ----
                R = wk.tile([128, 288], F16, tag="R", bufs=1, name=f"R{l}")
                R3 = R.rearrange("p (a s) -> p a s", s=12)
                for b in range(BL):
                    row_xr = 64 * b          # px rows: g = 2b + ri
                    row_xi = 64 * b + 32
                    for par in range(2):
                        out_r0 = 0 if par == 0 else 64
                        out_i0 = 32 if par == 0 else 96
                        # top + a=12 (direct): m = 23a + 2q + par, a in [0,12]
                        nc.vector.tensor_copy(
                            _cap(R3, out_r0, 32, [[12, 13], [2, 6]], b),
                            _cap(px, row_xr, 32, [[23, 13], [2, 6]], par))
                        nc.vector.tensor_copy(
                            _cap(R3, out_i0, 32, [[12, 13], [2, 6]], b),
                            _cap(px, row_xi, 32, [[23, 13], [2, 6]], par))
                        # bot bulk (conj): a in [13,24), in col 576-23a-2q-par
                        nc.vector.tensor_copy(
                            _cap(R3, out_r0, 32, [[12, 11], [2, 6]], 156 + b),
                            _cap(px, row_xr, 32, [[-23, 11], [-2, 6]], 277 - par))
                        nc.vector.tensor_scalar(
                            _cap(R3, out_i0, 32, [[12, 11], [2, 6]], 156 + b),
                            _cap(px, row_xi, 32, [[-23, 11], [-2, 6]], 277 - par),
                            -1.0, None, op0=ALU.mult)
                        # fixups: s = a-12 (P-columns, direct, xi positive)
                        cnt = 5 if par == 0 else 6
                        s0 = 2 if par == 0 else 1
                        o_off = 13 * s0 + 144 - par + b
                        i_off = 288 + s0 - 1
                        nc.vector.tensor_copy(
                            _cap(R3, out_r0, 32, [[26, cnt]], o_off),
                            _cap(px, row_xr, 32, [[2, cnt]], i_off))
                        nc.vector.tensor_copy(
                            _cap(R3, out_i0, 32, [[26, cnt]], o_off),
                            _cap(px, row_xi, 32, [[2, cnt]], i_off))

                # ---- mode mix: 288 64x64 matmuls (even/odd quadrants),
                # with the pm->frs extraction split in two so the first half
                # runs on DVE while the PE finishes the second half ----
                pm = wkps.tile([128, 288], F32, tag="pm", bufs=1, name=f"pm{l}")
                frs = [wk.tile([C, NWP], F32, tag=f"frs{b}", name=f"frs{l}_{b}")
                       for b in range(BL)]
                fis = [wk.tile([C, NWP], F32, tag=f"fis{b}", name=f"fis{l}_{b}")
                       for b in range(BL)]
                frx = [wk.tile([C, NWP], F32, tag=f"frx{b}", name=f"frx{l}_{b}")
                       for b in range(BL)]
                fix = [wk.tile([C, NWP], F32, tag=f"fix{b}", name=f"fix{l}_{b}")
                       for b in range(BL)]

                def mode_mix_half(hh):
                    for qq in range(4 * hh, 4 * hh + 4):
                        sl = mslab[(8 * l + qq) % 2]
                        for rl in range(18):
                            r = 18 * qq + rl
                            wcols = slice(64 * rl, 64 * rl + 64)
                            nc.tensor.matmul(pm[0:64, 2 * r:2 * r + 2],
                                             sl[0:64, wcols],
                                             R[0:64, 2 * r:2 * r + 2],
                                             start=True, stop=True,
                                             tile_position=(0, 0))
                            nc.tensor.matmul(pm[64:128, 2 * r:2 * r + 2],
                                             sl[64:128, wcols],
                                             R[64:128, 2 * r:2 * r + 2],
                                             start=True, stop=True,
                                             tile_position=(64, 64))
                        g_next = 8 * l + qq + 2
                        if g_next < 32:
                            mm_dma(g_next)

                def extract_half(hh):
                    # even u from pm rows 0:32 (or) / 32:64 (oi), odd u from
                    # 64:96 / 96:128; half hh covers mode columns 144h:144h+144
                    off = 144 * hh
                    for b in range(BL):
                        nc.vector.tensor_copy(
                            _cap(frs[b], 0, 32, [[2, 72]], off),
                            _cap(pm, 0, 32, [[2, 72]], off + b))
                        nc.vector.tensor_copy(
                            _cap(frs[b], 0, 32, [[2, 72]], off + 1),
                            _cap(pm, 64, 32, [[2, 72]], off + b))
                        nc.vector.tensor_copy(
                            _cap(fis[b], 0, 32, [[2, 72]], off),
                            _cap(pm, 32, 32, [[2, 72]], off + b))
                        nc.vector.tensor_copy(
                            _cap(fis[b], 0, 32, [[2, 72]], off + 1),
                            _cap(pm, 96, 32, [[2, 72]], off + b))
                        # partner-coefficient slabs; j rows 1:6 draw only on
                        # the first 144 columns, 6:12 only on the second
                        j0, j1 = (1, 6) if hh == 0 else (6, 12)
                        for (dst, src) in ((frx[b], frs[b]), (fix[b], fis[b])):
                            d3 = dst[:, 0:288].rearrange("p (j i) -> p j i", i=24)
                            s3 = src[:, 0:288].rearrange("p (j i) -> p j i", i=24)
                            nc.vector.tensor_copy(d3[:, j0:j1, 1:12],
                                                  s3[:, j0:j1, 0:11])
                            nc.vector.tensor_copy(d3[:, j0:j1, 13:24],
                                                  s3[:, j0:j1, 12:23])
                            nc.vector.tensor_copy(d3[:, j0:j1, 0:1],
                                                  s3[:, j0:j1, 23:24])
                            if hh == 1:
                                # P columns: partner col 24*(23-j2)+11
                                nc.vector.tensor_copy(
                                    dst[:, 288:299],
                                    s3[:, 11:0:-1, 11:12].rearrange(
                                        "p j i -> p (j i)"))
                        if hh == 1:
                            nc.vector.tensor_scalar(fix[b][:, 288:299],
                                                    fix[b][:, 288:299],
                                                    -1.0, None, op0=ALU.mult)

                for b in range(BL):
                    nc.vector.memset(frs[b][:, 288:NWP], 0.0)
                    nc.vector.memset(fis[b][:, 288:NWP], 0.0)
                    nc.vector.memset(frx[b][:, 0:24], 0.0)
                    nc.vector.memset(frx[b][:, 299:NWP], 0.0)
                    nc.vector.memset(fix[b][:, 0:24], 0.0)
                    nc.vector.memset(fix[b][:, 299:NWP], 0.0)
                    # cols 24j+12 (j>=1) have no partner; they stay zero
                    nc.vector.memset(_cap(frx[b], 0, 32, [[24, 11]], 36), 0.0)
                    nc.vector.memset(_cap(fix[b], 0, 32, [[24, 11]], 36), 0.0)
                mode_mix_half(0)
                extract_half(0)
                mode_mix_half(1)
                extract_half(1)

                # ---- A^T / B^T via accumulate transposes ----
                cw3 = [128, 128, 48]
                AT = [[wk.tile([cw3[ch], C], F16, tag=f"AT{b}_{ch}",
                               name=f"AT{l}_{b}_{ch}") for ch in range(3)]
                      for b in range(BL)]
                BT = [[wk.tile([cw3[ch], C], F16, tag=f"BT{b}_{ch}",
                               name=f"BT{l}_{b}_{ch}") for ch in range(3)]
                      for b in range(BL)]
                for b in range(BL):
                    for ch in range(3):
                        cw_ = cw3[ch]
                        csl = slice(128 * ch, 128 * ch + cw_)
                        for di, (dstt, s_dir, s_flp) in enumerate(
                                ((AT[b][ch], frs[b], frx[b]),
                                 (BT[b][ch], fis[b], fix[b]))):
                            pc = wkps.tile([128, C], F32, tag="pc", bufs=1,
                                           name=f"pc{l}_{b}_{ch}_{di}")
                            nc.tensor.matmul(pc[0:cw_, :], s_dir[:, csl], is32_t[:],
                                             start=True, stop=False,
                                             is_transpose=True)
                            nc.tensor.matmul(pc[0:cw_, :], s_flp[:, csl], js32_t[:],
                                             start=False, stop=True,
                                             is_transpose=True)
                            # transpose-mode rhs magnitudes are not applied;
                            # the 1/32 coefficient scale happens here
                            nc.vector.tensor_scalar(dstt[:], pc[0:cw_, :],
                                                    1.0 / 32.0, None, op0=ALU.mult)

                # ---- inverse NUDFT + conv + activation (+ next-layer fwd) ----
                last = (l == 3)
                for c8 in range(8):
                    cols = slice(512 * c8, 512 * (c8 + 1))
                    pi_ = wkps.tile([64, 512], F32, tag="pinv", bufs=2,
                                    name=f"pinv{l}_{c8}")
                    for b in range(BL):
                        sl = pi_[32 * b:32 * (b + 1), :]
                        tp = (0, 32 * b)
                        nc.tensor.matmul(sl, AT[b][0][:], vinv_ap(b, 0, cols),
                                         start=True, stop=False, tile_position=tp)
                        nc.tensor.matmul(sl, AT[b][1][:], vinv_ap(b, 1, cols),
                                         start=False, stop=False, tile_position=tp)
                        nc.tensor.matmul(sl, AT[b][2][:], vinv_ap(b, 2, cols),
                                         start=False, stop=False, tile_position=tp)
                        nc.tensor.matmul(sl, BT[b][0][:], vinv_ap(b, 3, cols),
                                         start=False, stop=False, tile_position=tp)
                        nc.tensor.matmul(sl, BT[b][1][:], vinv_ap(b, 4, cols),
                                         start=False, stop=False, tile_position=tp)
                        nc.tensor.matmul(sl, BT[b][2][:], vinv_ap(b, 5, cols),
                                         start=False, stop=False, tile_position=tp)
                        nc.tensor.matmul(sl, cwt_t[l][:], h[b][:, cols],
                                         start=False, stop=True, tile_position=tp)
                    for b in range(BL):
                        nc.scalar.activation(
                            h[b][:, cols], pi_[32 * b:32 * (b + 1), :],
                            AF.Identity if last else AF.Gelu,
                            bias=cb_t[l][:, :])
                    if not last:
                        fwd_chunk(l + 1, c8, wk, wkps)
                    else:
                        # ---- head: fc1 + gelu + fc2, fused per chunk ----
                        for b in range(BL):
                            pg = wkps.tile([128, 512], F32, tag="pg", bufs=1,
                                           name=f"pg{b}_{c8}")
                            nc.tensor.matmul(pg[:], fc1w_t[:], h[b][:, cols],
                                             start=True, stop=True)
                            g = wk.tile([128, 512], F16, tag="g", bufs=2,
                                        name=f"g{b}_{c8}")
                            nc.scalar.activation(g[:], pg[:], AF.Gelu,
                                                 bias=fc1b_t[:, :])
                            py = wkps.tile([1, 512], F32, tag="py", bufs=1,
                                           name=f"py{b}_{c8}")
                            nc.tensor.matmul(py[:], fc2w_t[:], g[:],
                                             start=True, stop=True)
                            ys = wk.tile([1, 512], F16, tag="ys", bufs=1,
                                         name=f"ys{b}_{c8}")
                            nc.scalar.activation(ys[:], py[:], AF.Copy)
                            nc.sync.dma_start(y_d[b:b + 1, cols], ys[:])

    nc.compile()
    return nc


# --------------------------------------------------------------------------
# host marshaling
# --------------------------------------------------------------------------
def _marshal(pos, fc0_w, fc0_b, sw1r, sw1i, sw2r, sw2i, cw, cb,
             fc1_w, fc1_b, fc2_w, fc2_b):
    xp = (pos[:, :, 0] - pos[:, :, 0].min()).astype(np.float64)
    yp = (pos[:, :, 1] - pos[:, :, 1].min()).astype(np.float64)
    sx = np.float64(np.float32(6.28) / np.float32(xp.max()))
    sy = np.float64(np.float32(6.28) / np.float32(yp.max()))
    kx = np.concatenate([np.arange(MODES), np.arange(-MODES, 0)]).astype(np.float64)
    ky = np.concatenate([np.arange(MODES), np.arange(-(MODES - 1), 0)]).astype(np.float64)

    def wrap(v):
        return v - 2 * np.pi * np.round(v / (2 * np.pi))

    ck = np.zeros((B, 50, N), np.float32)
    ck[:, 0, :] = xp.astype(np.float32)
    ck[:, 1, :] = yp.astype(np.float32)
    for i in range(24):
        ck[:, 2 + i, :] = wrap(kx[i] * sx * xp).astype(np.float32)
    for j in range(23):
        ck[:, 26 + j, :] = wrap(ky[j] * sy * yp).astype(np.float32)
    ck[:, 49, :] = 1.0

    worder = _w_rows()
    # selT [50, 608]: cols 0:304 = -phase (Vi, sin(-theta)); cols 304:608 =
    # +phase (Vr, cos); the pi/2 cos shift is folded into the constant row so
    # one range-wrap covers every column.
    selT = np.zeros((50, 608), np.float32)
    for w, m in enumerate(worder):
        i, j = m % 24, m // 24
        selT[2 + i, w] = -1.0
        selT[26 + j, w] = -1.0
        selT[2 + i, 304 + w] = 1.0
        selT[26 + j, 304 + w] = 1.0
    selT[49, 0:304] = 0.0
    selT[49, 304:608] = np.pi / 2

    # mode-mix weights, dense augmented 64x64 blocks, even/odd halves
    mmw = np.zeros((4, 128, 9216), np.float16)
    for l in range(4):
        w1 = sw1r[l].astype(np.float64) + 1j * sw1i[l].astype(np.float64)
        w2 = sw2r[l].astype(np.float64) + 1j * sw2i[l].astype(np.float64)
        for u in range(288):
            a, s = u // 12, u % 12
            wm = w1[:, :, a, s] if a < 12 else w2[:, :, a - 12, s]
            wr = wm.real.astype(np.float16)
            wi = wm.imag.astype(np.float16)
            r, par = u // 2, u % 2
            blk = np.zeros((64, 64), np.float16)
            blk[0:32, 0:32] = wr
            blk[32:64, 0:32] = -wi
            blk[0:32, 32:64] = wi
            blk[32:64, 32:64] = wr
            mmw[l, 64 * par:64 * par + 64, 64 * r:64 * (r + 1)] = blk

    cwt = np.ascontiguousarray(cw.transpose(0, 2, 1)).astype(np.float16)  # [l, c_in, c_out]
    cbm = cb.reshape(4, C, 1).astype(np.float32)

    eye = np.eye(C, dtype=np.float32)
    args = dict(
        selT=selT,
        fc0w=fc0_w.astype(np.float32), fc0b=fc0_b.reshape(C, 1).astype(np.float32),
        mmw=mmw, cwt=cwt, cb=cbm,
        fc1w=fc1_w.astype(np.float16), fc1b=fc1_b.reshape(128, 1).astype(np.float32),
        fc2w=fc2_w.reshape(128, 1).astype(np.float16),
        i64=eye.astype(np.float16),
        i128=np.eye(128, dtype=np.float16),
        is32=eye.astype(np.float32),
        js32=eye[::-1].copy().astype(np.float32),
    )
    return ck, args


def kernel(**inputs):
    pos = np.asarray(inputs['pos'])
    ck, shared = _marshal(**{k: np.asarray(v) for k, v in inputs.items()})

    if 'nc' not in _CACHE:
        _CACHE['nc'] = _build_program()
    nc = _CACHE['nc']

    in_maps = []
    for core in range(NCORES):
        m = dict(shared)
        m['ck'] = ck[BL * core:BL * (core + 1)]
        in_maps.append(m)

    res = run_bass_kernel_spmd(nc, in_maps, list(range(NCORES)), trace=TRACE)
    _CACHE['last_results'] = res

    fc2_b = np.asarray(inputs['fc2_b']).astype(np.float32)
    out = np.zeros((B, N, 1), np.float32)
    for core in range(NCORES):
        out[BL * core:BL * (core + 1), :, 0] = res.results[core]['y'].astype(np.float32)
    out += fc2_b.reshape(1, 1, 1)
    return out
